# revision 72
# baseline (speedup 1.0000x reference)
"""Trainium2 Bass kernel for nn_KPLoss_377957122199 (v3, vector-lean).

loss = 1*CE + 4*smoothL1(kp) + 5*smoothL1(Procrustes rot residual)
     + 6*smoothL1(section-center diff)

Data-parallel over 8 cores (batch 8192 -> 1024/core). v3 design notes:
  * keypoints live in ONE [128, (d,k,s160)] bf16 tile per tensor (pred|gt
    packed side by side) so every phase-1/3 op is a full-width, long-run,
    2x-mode DVE op.  Products are emitted as two k-half tiles whose
    elementwise add IS tree level 1.
  * point sums (k-trees over raw keypoints) run on the idle GpSimd engine;
    phase-3's (gt + v) runs there too, overlapped with DVE products.
  * polar: 2 frob-scaled Newton iterations (optional plain Newton tail via
    TAIL_NEWTON; off -> rel ~7e-3 vs gate 2e-2, saves ~6.5us); reciprocals
    and powers go through batched Ln/Exp ACT ops in log domain, so the DVE
    does only small bf16 tensor ops.  Guards: |det| clamp, +-0.5 sign fold.
  * CE as in v2: fp8 logits shipped twice (n-major for the ONEHOT custom
    DVE gather; flat [100,4096] so a block-ones matmul reduces NS=20 on
    partitions into PSUM), ln reads PSUM packed 4 chunks/ACT op.
  * custom DVE ops: SL1_DIFF (fused smooth-L1-sum of (in0-in1)) and
    ONEHOT_DOT (sum_t logits[y_t,t] via PageIdx compare), 1 op per use.
"""

import sys
for _p in ("/opt/trn_rl_repo", "/root/.axon_site/_ro/trn_rl_repo"):
    if _p not in sys.path:
        sys.path.insert(0, _p)

from contextlib import ExitStack
from operator import add as _add_op

import numpy as np
import ml_dtypes

import concourse.bass as bass
import concourse.bacc as bacc
import concourse.mybir as mybir
import concourse.tile as tile
from concourse.bass_utils import run_bass_kernel_spmd

# ---- custom DVE ops (registered at import) --------------------------------
import concourse.dve_ops as dve_ops
from concourse.dve_ops import DveOp, OPS
from concourse.dve_spec import (
    C0, C1, C2, PageIdx, Spec, Src0, Src1, Zero,
    _has_src1, eq, lower, maxx, minn, select,
)
from concourse.dve_uop import DveOpSpec


def _sl1_ref(in0, in1, s0, s1, imm2):
    d = in0.astype(np.float32) - in1.astype(np.float32)
    t = np.clip(d, s0, s1)
    return (d - imm2 * t) * t


def _oh_ref(in0, in1, s0, s1, imm2):
    raise NotImplementedError


def _register(name, spec, subdim):
    if name in dve_ops._SUB_OPCODE_FOR_NAME:
        return next(o for o in OPS if o.name == name)
    row = dve_ops._CUSTOM_DVE_ROW_BASE + len(OPS)
    assert row < 0x20
    op = DveOp(name, spec, subdim=subdim, uops_sha={})
    for ver in ("v3", "v4"):
        s = DveOpSpec(name=name, opcode=row, uops=lower(spec, ver=ver),
                      rd1_en=_has_src1(spec))
        op.uops_sha[ver] = s.sha(ver)
    OPS.append(op)
    dve_ops._SUB_OPCODE_FOR_NAME[name] = row
    return op


_d = Src0 - Src1
_t = minn(maxx(_d, C0), C1)
SL1_DIFF = _register("SL1_DIFF", Spec(body=(_d - _t * C2) * _t, accum=_add_op,
                                      reference=_sl1_ref), subdim=False)
_pg = PageIdx(C0, C1)
ONEHOT_DOT = _register("ONEHOT_DOT",
                       Spec(body=select(eq(Src1, _pg), Src0, Zero),
                            accum=_add_op, reference=_oh_ref), subdim=True)

# ---- hand-written 2x_1p uop program for ONEHOT_DOT ------------------------
# The 1x program (3 states: init / steady / page-step) uses dp blocks 0-3:
#   dp0 carries the page counter (CURR_ALU_OUT temporal; +C1 in the step
#   state), dp1 IS_EQ(label, page), dp2 SELECT(zero, logit), dp3 adds the
#   selected value into the stage-local accumulator (CURR_ALU_OUT) and
#   captures the select into d0 for the WR0_LO output.
# The 2x variant processes the packed pair: extra input lanes route
# SRC_0_HI / SRC_1_HI; dp3/dp4 replicate IS_EQ/SELECT for the HI element,
# dp5 sums the LO+HI selects, dp6 accumulates the pair-sum, and WR0_HI
# emits the HI select from d1.  Lanes in the steady/step states:
#   d0: label_lo (consumed at dp1), then sel_lo from dp3 on
#   d1: zero, then sel_hi from dp5 on      d2: logit_lo
#   d3: C0 (dead), then logit_hi from dp0  d4: C1 (step), then page from dp1
#   d5: label_hi
ONEHOT_2X = True
TAIL_NEWTON = False             # 2 scaled iters only; rel ~7e-3 (gate 2e-2)


def _onehot_uops_2x(base):
    import copy
    from concourse.dve_uop import AluInp, DelayInp, InpSel, OutPath, OutSel
    from concourse.dve_spec import AluOp as _SpecAluOp  # noqa: F401
    from concourse.dve_uop import AluOp as _UopAluOp
    PP, PA = DelayInp.PREV_DELAY, DelayInp.PREV_ALU_OUT
    u2x = [copy.deepcopy(u) for u in base]
    for si in (1, 2):
        u = u2x[si]
        u.inp[0], u.inp_enable[0] = InpSel.SRC_0_HI, 1
        u.inp[6], u.inp_enable[6] = InpSel.SRC_1_HI, 1
        dp = u.datapath_config
        # dp0: page counter (op kept) + capture SRC_0_HI (lane0) into d3,
        # pass label_hi through d5
        dp[0].delay[3], dp[0].delay_enable[3] = PA, 1
        dp[0].delay[5], dp[0].delay_enable[5] = PP, 1
        # dp1: IS_EQ lo (kept) + capture page into d4
        dp[1].delay[4], dp[1].delay_enable[4] = PA, 1
        dp[1].delay[3], dp[1].delay_enable[3] = PP, 1
        dp[1].delay[5], dp[1].delay_enable[5] = PP, 1
        # dp2: SELECT lo (kept); keep lanes flowing
        dp[2].delay[3], dp[2].delay_enable[3] = PP, 1
        dp[2].delay[4], dp[2].delay_enable[4] = PP, 1
        dp[2].delay[5], dp[2].delay_enable[5] = PP, 1
        # dp3: accumulate sel_lo temporally (EXACTLY as the 1x program:
        # ADD(CURR_ALU_OUT, PREV_ALU_OUT), a-flop inject, capture sel_lo d0)
        # -- dp3 base is already that; also pass d1 (zero), d3 (logit_hi),
        # d4 (page), d5 (label_hi)
        dp[3].delay[1], dp[3].delay_enable[1] = PP, 1
        dp[3].delay[3], dp[3].delay_enable[3] = PP, 1
        dp[3].delay[4], dp[3].delay_enable[4] = PP, 1
        dp[3].delay[5], dp[3].delay_enable[5] = PP, 1
        # dp4: IS_EQ hi; capture acc_lo into d2
        dp[4].op = _UopAluOp.IS_EQ
        dp[4].alu_src0 = AluInp.PREV_DELAY_5
        dp[4].alu_src1 = AluInp.PREV_DELAY_4
        dp[4].delay[0], dp[4].delay_enable[0] = PP, 1
        dp[4].delay[1], dp[4].delay_enable[1] = PP, 1
        dp[4].delay[2], dp[4].delay_enable[2] = PA, 1
        dp[4].delay[3], dp[4].delay_enable[3] = PP, 1
        # dp5: SELECT hi
        dp[5].op = _UopAluOp.SELECT
        dp[5].alu_src0 = AluInp.PREV_DELAY_1
        dp[5].alu_src1 = AluInp.PREV_DELAY_3
        dp[5].delay[0], dp[5].delay_enable[0] = PP, 1
        dp[5].delay[2], dp[5].delay_enable[2] = PP, 1
        # dp6: accumulate sel_hi temporally; capture sel_hi into d1
        dp[6].op = _UopAluOp.ADD
        dp[6].alu_src0 = AluInp.CURR_ALU_OUT
        dp[6].alu_src1 = AluInp.PREV_ALU_OUT
        dp[6].delay[0], dp[6].delay_enable[0] = PP, 1
        dp[6].delay[1], dp[6].delay_enable[1] = PA, 1
        dp[6].delay[2], dp[6].delay_enable[2] = PP, 1
        # dp7: total = acc_lo + acc_hi.  The per-lane accumulator register is
        # not reachable from the 2x uop table (hw READ_ACCUMULATOR returns
        # junk), so the running total itself is emitted on WR0_LO: the last
        # even element of the dump holds the final sum.
        dp[7].op = _UopAluOp.ADD
        dp[7].alu_src0 = AluInp.PREV_DELAY_2
        dp[7].alu_src1 = AluInp.PREV_ALU_OUT
        dp[7].delay[0], dp[7].delay_enable[0] = PP, 1
        dp[7].delay[1], dp[7].delay_enable[1] = PP, 1
        u.out[OutPath.WR0_LO] = OutSel.ALU_OUT
        u.out_enable[OutPath.WR0_LO] = 1
        u.out[OutPath.WR0_HI] = OutSel.DELAY_1
        u.out_enable[OutPath.WR0_HI] = 1
    return u2x


def _install_onehot_2x():
    from concourse.dve_table_gen import dve_ver_for
    ver = dve_ver_for("TRN2")
    base = lower(ONEHOT_DOT.spec, ver=ver)
    spec = DveOpSpec(
        name="ONEHOT_DOT",
        opcode=dve_ops.get_dve_sub_opcode("ONEHOT_DOT"),
        uops=base,
        uops_2x=_onehot_uops_2x(base),
        perf_max=1,
        rd1_en=True,
    )
    for u in spec.uops_2x:
        u.validate(ver)
    dve_ops._COMPILE_CACHE[("ONEHOT_DOT", ver)] = spec


if ONEHOT_2X:
    _install_onehot_2x()

FP32 = mybir.dt.float32
BF16 = mybir.dt.bfloat16
FP8 = mybir.dt.float8e4
AX = mybir.AxisListType
OP = mybir.AluOpType
AF = mybir.ActivationFunctionType

N_CORES = 8
B, K, NS, SEC = 8192, 400, 20, 20
S = K // SEC                    # 20 sections / sample
BC = B // N_CORES               # 1024 samples / core
F = 160                         # sections per partition ("sall" = (chunk, s))
NCH_CE = 20                     # CE chunks
TOKC = BC * K // NCH_CE         # 20480 tokens / CE chunk
T_CE = TOKC // 128              # 160 tokens / partition (n-major layout)
FFL = TOKC * NS // 100          # 4096 cols in flat [100, .] layout

N_ITER = 2                      # scaled-Newton polar iterations
OH2X_CUT = 20                   # chunks >= this use fp8 + 1x ONEHOT
LN_HALF = float(np.log(0.5))
LN_DET_MIN = float(np.log(1e-6))

# acc column map
C_LSE = 0                       # 5 cols (groups of 4 chunks; rows 32q+0..4)
C_LY = C_LSE + 5                # 20 cols
C_KP = C_LY + NCH_CE            # 1
C_ROT = C_KP + 1                # 1
C_CENT = C_ROT + 1              # 1
C_LYX = C_CENT + 1              # 4 (second halves of split chunks 0..3)
NACC = C_LYX + 4


def _emit(ctx, tc, aps):
    nc = tc.nc
    pg, lgn, lgn8, lgf, lb, ob, out = (aps[k] for k in
                                 ("pg", "lgn", "lgn8", "lgf", "lb", "ob", "out"))

    pers = ctx.enter_context(tc.tile_pool(name="pers", bufs=1))
    scr = ctx.enter_context(tc.tile_pool(name="scr", bufs=1))
    cep = tc.alloc_tile_pool(name="ce", bufs=3)
    exp_pool = tc.alloc_tile_pool(name="exp", bufs=2)
    psp = tc.alloc_tile_pool(name="ps", bufs=1, space="PSUM")

    acc = pers.tile([128, NACC], FP32, tag="acc", name="acc")
    oneblk = pers.tile([100, 5], BF16, tag="oneblk", name="oneblk")
    nc.sync.dma_start(oneblk[:], ob)
    lnhalf = pers.tile([128, 1], FP32, tag="lnhalf", name="lnhalf")
    nc.gpsimd.memset(lnhalf[:], LN_HALF)

    # keypoints: pred at [:, 0:9600], gt at [:, 9600:19200]; layout (d,k,f)
    pgb = pers.tile([128, 2 * 9600], BF16, tag="pgb", name="pgb")
    p_all = pgb[:, 0:9600]
    g_all = pgb[:, 9600:19200]

    # A66 holds the polar 5x5-duplicated state; H == its X quadrant, so the
    # phase-1 tree writes land directly where the polar iteration reads.
    A66 = pers.tile([128, 25 * F], BF16, tag="A66", name="A66")
    A = A66[:].rearrange("p (a b f) -> p a b f", a=5, b=5)
    Hv = A[:, 0:3, 0:3]                                  # [128,3,3,F]
    spg = pers.tile([128, 2 * 3 * F], BF16, tag="spg", name="spg")  # [s,d,f] sums
    spgs = pers.tile([128, 2 * 3 * F], BF16, tag="spgs", name="spgs")  # /SEC
    R = pers.tile([128, 9 * F], BF16, tag="R", name="R")        # [i,j,f]
    v = pers.tile([128, 3 * F], BF16, tag="v", name="v")        # [j,f]

    # ---------------- cross entropy ----------------
    psum = psp.tile([128, FFL], FP32, tag="mm", name="mm")

    def ce_chunk(c):
        # early chunks: bf16 logits + 2x ONEHOT (fast, DVE-bound window);
        # late chunks: fp8 logits + 1x ONEHOT (halves their DMA in the
        # DMA-feed-bound tail, where the DVE idles anyway)
        two_x = ONEHOT_2X and c < OH2X_CUT
        split = two_x and c < 4          # class-halved gather: earlier start
        if two_x:
            lgnc = cep.tile([128, NS * T_CE], BF16, tag="lgn", name="lgn")
            if split:
                he = (NS // 2) * T_CE
                nc.sync.dma_start(lgnc[:, 0:he], lgn[c][:, 0:he])
                nc.sync.dma_start(lgnc[:, he:2 * he], lgn[c][:, he:2 * he])
            else:
                nc.sync.dma_start(lgnc[:], lgn[c])
        else:
            lgnc = cep.tile([128, NS * T_CE], FP8, tag="lgn8", name="lgn8")
            nc.sync.dma_start(lgnc[:], lgn8[c - OH2X_CUT])
        lbc = cep.tile([128, T_CE], BF16, tag="lbc", name="lbc")
        nc.sync.dma_start(lbc[:], lb[c])
        lgfc = cep.tile([100, FFL], FP8, tag="lgf", name="lgf")
        nc.sync.dma_start(lgfc[:], lgf[c])

        # l_y: one custom op.  In 2x mode the op streams its running total on
        # WR0_LO; the final sum sits in the last even dump element and a tiny
        # copy moves it into the acc column.
        dmp = scr.tile([128, NS * T_CE], BF16, tag="dmpce", name="dmpce")
        if split:
            he = (NS // 2) * T_CE
            for h in range(2):
                col = C_LY + c if h == 0 else C_LYX + c
                oh = nc.vector._custom_dve(
                    ONEHOT_DOT,
                    out=dmp[:, h * he:(h + 1) * he].rearrange(
                        "p (n t) -> p n t", n=NS // 2),
                    in0=lgnc[:, h * he:(h + 1) * he].rearrange(
                        "p (n t) -> p n t", n=NS // 2),
                    in1=lbc[:].unsqueeze(1).broadcast_to(
                        [128, NS // 2, T_CE]),
                    s0=float(h * (NS // 2)), s1=1.0, accum_out=None)
                oh.ins.perf_max = 1
                nc.vector.tensor_copy(acc[:, col:col + 1],
                                      dmp[:, (h + 1) * he - 2:(h + 1) * he - 1])
        else:
            oh = nc.vector._custom_dve(
                ONEHOT_DOT,
                out=dmp[:].rearrange("p (n t) -> p n t", n=NS),
                in0=lgnc[:].rearrange("p (n t) -> p n t", n=NS),
                in1=lbc[:].unsqueeze(1).broadcast_to([128, NS, T_CE]),
                s0=0.0, s1=1.0,
                accum_out=None if two_x else acc[:, C_LY + c:C_LY + c + 1])
            if two_x:
                oh.ins.perf_max = 1
                nc.vector.tensor_copy(acc[:, C_LY + c:C_LY + c + 1],
                                      dmp[:, NS * T_CE - 2:NS * T_CE - 1])

        # lse: exp (scalar) -> block-ones matmul (PE) -> ln on packed PSUM
        ex = exp_pool.tile([100, FFL], BF16, tag="ex", name="ex")
        nc.scalar.activation(ex[:], lgfc[:], AF.Exp)
        q = c % 4
        for h in range(FFL // 512):
            nc.tensor.matmul(
                psum[32 * q:32 * q + 5, h * 512:(h + 1) * 512],
                oneblk[:], ex[:, h * 512:(h + 1) * 512],
                start=True, stop=True, tile_position=(0, 32 * q))
        if q == 3:
            g = c // 4
            lnd = scr.tile([101, FFL], BF16, tag="lnd", name="lnd")
            nc.scalar.activation(lnd[:], psum[0:101, :], AF.Ln,
                                 accum_out=acc[0:101, C_LSE + g:C_LSE + g + 1])

    # ---------------- phase 1: keypoints ----------------
    def p1():
        wk = tc.alloc_tile_pool(name="wk1", bufs=1)
        # keypoints ship k-half-major: region layout (kh2, d3, k10, f160), so
        # the first product op depends only on the first half of the DMA.
        p4 = p_all.rearrange("p (kh d kf) -> p kh d kf", kh=2, d=3)
        g4 = g_all.rearrange("p (kh d kf) -> p kh d kf", kh=2, d=3)

        # products per k-half, each tree-reduced k10 -> k5 immediately, then
        # the two k5 tiles merge.  A2a ends up holding sum over each k-5-group.
        A2a = wk.tile([128, 9 * 5 * F], BF16, tag="A2a", name="A2a")
        A2av = A2a[:].rearrange("p (m k f) -> p m k f", m=9, k=5)
        A2b = wk.tile([128, 9 * 5 * F], BF16, tag="A2b", name="A2b")
        A2bv = A2b[:].rearrange("p (m k f) -> p m k f", m=9, k=5)
        for half, A2h in ((0, A2av), (1, A2bv)):
            T = wk.tile([128, 9 * 1600], BF16, tag="TT", name="TT")
            nc.vector.tensor_tensor(
                T[:].rearrange("p (i j kf) -> p i j kf", i=3, j=3),
                g4[:, half].unsqueeze(2).broadcast_to([128, 3, 3, 10 * F]),
                p4[:, half].unsqueeze(1).broadcast_to([128, 3, 3, 10 * F]),
                OP.mult)
            Tv = T[:].rearrange("p (m k f) -> p m k f", m=9, k=10)
            nc.vector.tensor_tensor(A2h, Tv[:, :, 0:5], Tv[:, :, 5:10], OP.add)
        nc.vector.tensor_tensor(A2a[:], A2a[:], A2b[:], OP.add)  # k5 sums
        D1 = wk.tile([128, 9 * 2 * F], BF16, tag="D1", name="D1")
        D1v = D1[:].rearrange("p (m k f) -> p m k f", m=9, k=2)
        nc.vector.tensor_tensor(D1v, A2av[:, :, 0:2], A2av[:, :, 2:4], OP.add)
        D2 = wk.tile([128, 9 * F], BF16, tag="D2", name="D2")
        D2v = D2[:].rearrange("p (m f) -> p m f", m=9)
        nc.vector.tensor_tensor(D2v, D1v[:, :, 0], D1v[:, :, 1], OP.add)
        nc.vector.tensor_tensor(
            Hv, D2[:].rearrange("p (i j f) -> p i j f", i=3, j=3),
            A2a[:].rearrange("p (i j k f) -> p i j k f", i=3, j=3, k=5)[:, :, :, 4],
            OP.add)

        # point sums over k for pred AND gt (k-tree, both tensors per op);
        # level 1 sums the two k-halves elementwise
        pgv = pgb[:].rearrange("p (s kh dkf) -> p s kh dkf", s=2, kh=2)
        PB1 = wk.tile([128, 9 * 1600], BF16, tag="TT", name="PB1")
        PB1v = PB1[:, 0:2 * 3 * 10 * F].rearrange(
            "p (s d k f) -> p s d k f", s=2, d=3, k=10)
        nc.vector.tensor_tensor(
            PB1v.rearrange("p s d k f -> p s (d k f)"),
            pgv[:, :, 0], pgv[:, :, 1], OP.add)
        PB2 = wk.tile([128, 2 * 3 * 5 * F], BF16, tag="PB2", name="PB2")
        PB2v = PB2[:].rearrange("p (s d k f) -> p s d k f", s=2, d=3, k=5)
        nc.vector.tensor_tensor(PB2v, PB1v[:, :, :, 0:5], PB1v[:, :, :, 5:10],
                                OP.add)
        PD1 = wk.tile([128, 2 * 3 * 2 * F], BF16, tag="PD1", name="PD1")
        PD1v = PD1[:].rearrange("p (s d k f) -> p s d k f", s=2, d=3, k=2)
        nc.vector.tensor_tensor(PD1v, PB2v[:, :, :, 0:2], PB2v[:, :, :, 2:4],
                                OP.add)
        PD2 = wk.tile([128, 2 * 3 * F], BF16, tag="PD2", name="PD2")
        PD2v = PD2[:].rearrange("p (s d f) -> p s d f", s=2, d=3)
        nc.vector.tensor_tensor(PD2v, PD1v[:, :, :, 0], PD1v[:, :, :, 1],
                                OP.add)
        nc.vector.tensor_tensor(
            spg[:].rearrange("p (s d f) -> p s d f", s=2, d=3),
            PD2v, PB2v[:, :, :, 4], OP.add)
        # scaled copies (sums/SEC) used by H-correction, v, and center loss
        nc.vector.tensor_scalar(spgs[:], spg[:], 1.0 / SEC, None, OP.mult)

        # H -= sg_i * (sp_j/SEC)
        spsv = spgs[:, 0:3 * F].rearrange("p (d f) -> p d f", d=3)
        sgv = spg[:, 3 * F:6 * F].rearrange("p (d f) -> p d f", d=3)
        Mt = wk.tile([128, 9 * 2 * F], BF16, tag="D1", name="M")
        M = Mt[:, 0:9 * F]
        nc.vector.tensor_tensor(
            M.rearrange("p (i j f) -> p i j f", i=3, j=3),
            sgv.unsqueeze(2).broadcast_to([128, 3, 3, F]),
            spsv.unsqueeze(1).broadcast_to([128, 3, 3, F]), OP.mult)
        nc.vector.tensor_tensor(
            Hv, Hv, M.rearrange("p (i j f) -> p i j f", i=3, j=3),
            OP.subtract)

        # keypoint smooth-L1 (one op) + center loss
        dmp = wk.tile([128, 9 * 1600], BF16, tag="TT", name="TTdmp")
        nc.vector._custom_dve(SL1_DIFF, out=dmp[:, 0:9600], in0=p_all,
                              in1=g_all, s0=-1.0, s1=1.0, imm2=0.5,
                              accum_out=acc[:, C_KP:C_KP + 1])
        dmpc = scr.tile([128, 3 * F], BF16, tag="dmpc", name="dmpc")
        nc.vector._custom_dve(SL1_DIFF, out=dmpc[:], in0=spgs[:, 0:3 * F],
                              in1=spgs[:, 3 * F:6 * F],
                              s0=-1.0, s1=1.0, imm2=0.5,
                              accum_out=acc[:, C_CENT:C_CENT + 1])
        wk.release()

    # ---------------- polar decomposition (batched 3x3, bf16) ----------------
    def polar(fillers=()):
        fillers = list(fillers)
        pol = tc.alloc_tile_pool(name="pol", bufs=1)
        X = Hv
        Cf = pol.tile([128, 9 * F], BF16, tag="cof", name="cof")
        C3v = Cf[:].rearrange("p (i j f) -> p i j f", i=3, j=3)
        T1 = pol.tile([128, 9 * F], BF16, tag="ct1", name="ct1")
        T2 = pol.tile([128, 9 * F], BF16, tag="ct2", name="ct2")
        SX = pol.tile([128, 9 * F], BF16, tag="sqX", name="sqX")
        q1 = pol.tile([128, 4 * F], BF16, tag="fq1", name="fq1")
        q2 = pol.tile([128, 2 * F], BF16, tag="fq2", name="fq2")
        P0 = pol.tile([128, 3 * F], BF16, tag="dp0", name="dp0")
        P0v = P0[:].rearrange("p (j f) -> p j f", j=3)
        det = pol.tile([128, F], BF16, tag="det", name="det")
        sgn05 = pol.tile([128, F], BF16, tag="sgn", name="sgn")
        # combined ACT operands: one Ln over [nx2 | nc2 | adet], one Exp over
        # [hzarg | twarg] -- 2 ACT round-trips per iteration instead of 5.
        lnin = pol.tile([128, 3 * F], BF16, tag="lnin", name="lnin")
        lnout = pol.tile([128, 3 * F], FP32, tag="lnout", name="lnout")
        exparg = pol.tile([128, 2 * F], FP32, tag="exparg", name="exparg")
        expout = pol.tile([128, 2 * F], BF16, tag="expout", name="expout")
        lnd = pol.tile([128, F], FP32, tag="lnd", name="lnd")
        t3 = pol.tile([128, F], FP32, tag="t3", name="t3")
        wz0 = pol.tile([128, F], BF16, tag="wz0", name="wz0")
        wz = pol.tile([128, F], BF16, tag="wz", name="wz")
        U1 = pol.tile([128, 9 * F], BF16, tag="u1", name="u1")
        U1v = U1[:].rearrange("p (i j f) -> p i j f", i=3, j=3)
        U2 = pol.tile([128, 9 * F], BF16, tag="u2", name="u2")
        U2v = U2[:].rearrange("p (i j f) -> p i j f", i=3, j=3)
        LN2 = float(np.log(2.0))
        chz = pers.tile([128, 1], FP32, tag="chz", name="chz")
        nc.gpsimd.memset(chz[:], -1.25 * LN2)

        def cof_det(adet_dst):
            # duplicate X -> A66 quadrants (on ACT), cofactors via shifted
            # views, det; adet = |det|/2 lands in adet_dst
            nc.scalar.copy(A[:, 0:3, 3:5], A[:, 0:3, 0:2])
            nc.scalar.copy(A[:, 3:5, :], A[:, 0:2, :])
            nc.vector.tensor_tensor(
                T1[:].rearrange("p (i j f) -> p i j f", i=3, j=3),
                A[:, 1:4, 1:4], A[:, 2:5, 2:5], OP.mult)
            nc.vector.tensor_tensor(
                T2[:].rearrange("p (i j f) -> p i j f", i=3, j=3),
                A[:, 1:4, 2:5], A[:, 2:5, 1:4], OP.mult)
            nc.vector.tensor_tensor(Cf[:], T1[:], T2[:], OP.subtract)
            nc.vector.tensor_tensor(P0v, A[:, 0, 0:3], C3v[:, 0], OP.mult)
            nc.vector.tensor_tensor(det[:].unsqueeze(1), P0v[:, 0:1],
                                    P0v[:, 1:2], OP.add)
            nc.vector.tensor_tensor(det[:], det[:], P0v[:, 2], OP.add)
            nc.vector.tensor_scalar(sgn05[:], det[:], 0.0, -0.5, OP.is_ge,
                                    OP.add)
            nc.vector.tensor_tensor(adet_dst, det[:], sgn05[:], OP.mult)

        def frob(dst, src4):
            # dst <- max(frob2(src4), 1e-12)   (a [128,F] slice AP)
            nc.vector.tensor_tensor(
                SX[:].rearrange("p (i j f) -> p i j f", i=3, j=3),
                src4, src4, OP.mult)
            sx = SX[:].rearrange("p (m f) -> p m f", m=9)
            q1v = q1[:].rearrange("p (m f) -> p m f", m=4)
            nc.vector.tensor_tensor(q1v, sx[:, 0:4], sx[:, 4:8], OP.add)
            q2v = q2[:].rearrange("p (m f) -> p m f", m=2)
            nc.vector.tensor_tensor(q2v, q1v[:, 0:2], q1v[:, 2:4], OP.add)
            nc.vector.tensor_tensor(dst.unsqueeze(1), q2v[:, 0:1],
                                    q2v[:, 1:2], OP.add)
            nc.vector.tensor_tensor(dst, dst, sx[:, 8], OP.add)
            nc.vector.tensor_scalar(dst, dst, 1e-12, None, OP.max)

        for it in range(N_ITER):
            frob(lnin[:, 0:F], X)    # overlaps the ACT dup-copies in cof_det
            cof_det(lnin[:, 2 * F:3 * F])
            frob(lnin[:, F:2 * F], C3v)
            nc.scalar.activation(lnout[:], lnin[:], AF.Ln)
            # lnd'' = max(ln(adet), LN_MIN-ln2) + ln2/2  (= clamped ln|det|
            # - ln2/2; the ln2 shifts from adet = |det|/2 fold into consts)
            nc.vector.tensor_scalar(lnd[:], lnout[:, 2 * F:3 * F],
                                    LN_DET_MIN - LN2, 0.5 * LN2,
                                    OP.max, OP.add)
            # t3'' = ln(nc2) - ln(nx2) - 2*lnd''
            nc.vector.tensor_tensor(t3[:], lnout[:, F:2 * F], lnout[:, 0:F],
                                    OP.subtract)
            nc.vector.scalar_tensor_tensor(t3[:], lnd[:], -2.0, t3[:],
                                           OP.mult, OP.add)
            # hzarg = t3''/4 - 1.25*ln2 ; twarg = -t3''/4 - lnd'' - ln2/4
            nc.vector.scalar_tensor_tensor(
                exparg[:, 0:F], t3[:], 0.25,
                chz[:].broadcast_to([128, F]), OP.mult, OP.add)
            nc.vector.scalar_tensor_tensor(exparg[:, F:2 * F], t3[:], -0.25,
                                           lnd[:], OP.mult, OP.subtract)
            nc.vector.tensor_scalar(exparg[:, F:2 * F], exparg[:, F:2 * F],
                                    -0.25 * LN2, None, OP.add)
            nc.scalar.activation(expout[:], exparg[:], AF.Exp)
            nc.vector.tensor_tensor(wz[:], expout[:, F:2 * F], sgn05[:],
                                    OP.mult)
            # X = X*hz + C*wz
            hzb = expout[:, 0:F].unsqueeze(1).unsqueeze(1).broadcast_to(
                [128, 3, 3, F])
            wzb = wz[:].unsqueeze(1).unsqueeze(1).broadcast_to([128, 3, 3, F])
            nc.vector.tensor_tensor(U1v, X, hzb, OP.mult)
            nc.vector.tensor_tensor(U2v, C3v, wzb, OP.mult)
            nc.vector.tensor_tensor(X, U1v, U2v, OP.add)

        Rv = R[:].rearrange("p (i j f) -> p i j f", i=3, j=3)
        if TAIL_NEWTON:
            # plain Newton tail: R = 0.5*X + C * sgn/(2|det|)
            adet_t = lnin[:, 2 * F:3 * F]
            cof_det(adet_t)
            nc.scalar.activation(lnd[:], adet_t, AF.Ln, scale=2.0)
            nc.vector.tensor_scalar(lnd[:], lnd[:], LN_DET_MIN, None, OP.max)
            nc.scalar.activation(wz0[:], lnd[:], AF.Exp, scale=-1.0)
            nc.vector.tensor_tensor(wz[:], wz0[:], sgn05[:], OP.mult)
            wzb = wz[:].unsqueeze(1).unsqueeze(1).broadcast_to(
                [128, 3, 3, F])
            nc.vector.tensor_tensor(U2v, C3v, wzb, OP.mult)
            nc.vector.tensor_scalar(Rv, X, 0.5, None, OP.mult)
            nc.vector.tensor_tensor(R[:], R[:], U2[:], OP.add)
        else:
            nc.vector.tensor_copy(Rv, X)

        # v_j = sum_i (sp_i/SEC) R_ij - sg_j/SEC
        spsv = spgs[:, 0:3 * F].rearrange("p (d f) -> p d f", d=3)
        sgsv = spgs[:, 3 * F:6 * F].rearrange("p (d f) -> p d f", d=3)
        Pv = pol.tile([128, 9 * F], BF16, tag="Pv", name="Pv")
        Pvv = Pv[:].rearrange("p (i j f) -> p i j f", i=3, j=3)
        nc.vector.tensor_tensor(
            Pvv, spsv.unsqueeze(2).broadcast_to([128, 3, 3, F]), Rv, OP.mult)
        vv = v[:].rearrange("p (j f) -> p j f", j=3)
        nc.vector.tensor_tensor(vv, Pvv[:, 0], Pvv[:, 1], OP.add)
        nc.vector.tensor_tensor(vv, vv, Pvv[:, 2], OP.add)
        nc.vector.tensor_tensor(vv, vv, sgsv, OP.subtract)
        pol.release()

    # ---------------- phase 3: rotation residual ----------------
    def p3_segments():
        """Phase 3 as a list of ~5us DVE segments; the schedule interleaves
        them between the DMA-paced tail CE chunks so each segment covers one
        chunk's lgn transfer latency."""
        wk = tc.alloc_tile_pool(name="wk3", bufs=1)
        # keypoint layout (kh2, d3, k10, f); T/gv tiles use (j, kh, k10, f)
        p3v = p_all.rearrange("p (kh d k f) -> p d kh k f", kh=2, d=3, k=10)
        g3v = g_all.rearrange("p (kh d k f) -> p d kh k f", kh=2, d=3, k=10)
        Rv = R[:].rearrange("p (i j f) -> p i j f", i=3, j=3)
        vv = v[:].rearrange("p (j f) -> p j f", j=3)
        gv = wk.tile([128, 9600], BF16, tag="gv", name="gv")
        gvv = gv[:].rearrange("p (j kh k f) -> p j kh k f", j=3, kh=2, k=10)
        T0 = wk.tile([128, 9600], BF16, tag="r0", name="r0")
        Tt = [T0,
              wk.tile([128, 9600], BF16, tag="r1", name="r1"),
              wk.tile([128, 9600], BF16, tag="r2", name="r2")]

        def seg_gv():
            for kh in range(2):
                nc.vector.tensor_tensor(
                    gvv[:, :, kh], g3v[:, :, kh],
                    vv.unsqueeze(2).broadcast_to([128, 3, 10, F]), OP.add)

        def seg_prod(i):
            def run():
                Tv = Tt[i][:].rearrange("p (j kh k f) -> p j kh k f",
                                        j=3, kh=2, k=10)
                for kh in range(2):
                    nc.vector.tensor_tensor(
                        Tv[:, :, kh],
                        p3v[:, i, kh].unsqueeze(0 + 1).broadcast_to(
                            [128, 3, 10, F]),
                        Rv[:, i].unsqueeze(2).broadcast_to([128, 3, 10, F]),
                        OP.mult)
            return run

        def seg_add(i):
            def run():
                nc.vector.tensor_tensor(T0[:], T0[:], Tt[i][:], OP.add)
            return run

        def seg_sl1():
            dmp = wk.tile([128, 9600], BF16, tag="r1", name="r1dmp")
            nc.vector._custom_dve(SL1_DIFF, out=dmp[:], in0=T0[:], in1=gv[:],
                                  s0=-1.0, s1=1.0, imm2=0.5,
                                  accum_out=acc[:, C_ROT:C_ROT + 1])
            wk.release()

        return [seg_gv, seg_prod(0), seg_prod(1), seg_prod(2),
                seg_add(1), seg_add(2), seg_sl1]

    # ---------------- schedule ----------------
    # keypoints first (4 DMAs for queue spread); CE chunk DMAs flow on other
    # queues.  Polar is emitted early so its small ACT ops (ln/exp) are not
    # queued behind the bulk of the 3.7us CE exp ops; the remaining CE
    # chunks fill DVE after polar while ACT drains exps, and phase 3 ends
    # the DVE stream.
    ce_chunk(0)
    ce_chunk(1)
    for h in range(2):
        nc.sync.dma_start(pgb[:, h * 4800:(h + 1) * 4800],
                          pg[0][:, h * 4800:(h + 1) * 4800])
        nc.sync.dma_start(pgb[:, 9600 + h * 4800:9600 + (h + 1) * 4800],
                          pg[1][:, h * 4800:(h + 1) * 4800])
    for c in range(2, 12):
        ce_chunk(c)
    p1()
    polar()
    for c in range(12, NCH_CE):
        ce_chunk(c)
    for seg in p3_segments():
        seg()
    psp.release()
    exp_pool.release()
    cep.release()
    nc.sync.dma_start(out[:], acc[:])


_CACHE = {}


def _build():
    if "nc" in _CACHE:
        return _CACHE["nc"]
    nc = bacc.Bacc("TRN2", target_bir_lowering=False, debug=False,
                   enable_asserts=False, num_devices=N_CORES)
    aps = {
        "pg": nc.dram_tensor("pg", [2, 128, 9600], BF16, kind="ExternalInput").ap(),
        "lgn": nc.dram_tensor("lgn", [OH2X_CUT, 128, NS * T_CE],
                              BF16, kind="ExternalInput").ap(),
        "lgn8": nc.dram_tensor("lgn8", [max(1, NCH_CE - OH2X_CUT), 128,
                                        NS * T_CE], FP8,
                               kind="ExternalInput").ap(),
        "lgf": nc.dram_tensor("lgf", [NCH_CE, 100, FFL], FP8, kind="ExternalInput").ap(),
        "lb": nc.dram_tensor("lb", [NCH_CE, 128, T_CE], BF16, kind="ExternalInput").ap(),
        "ob": nc.dram_tensor("ob", [100, 5], BF16, kind="ExternalInput").ap(),
        "out": nc.dram_tensor("out", [128, NACC], FP32, kind="ExternalOutput").ap(),
    }
    with tile.TileContext(nc) as tc:
        with ExitStack() as ctx:
            with nc.allow_low_precision(reason="bf16 tree reduces; validated"):
                _emit(ctx, tc, aps)
    nc.compile()
    _CACHE["nc"] = nc
    return nc


def _shard_inputs(pred_keypoints, gt_keypoints, pred_section_logits, gt_section_label):
    bf = ml_dtypes.bfloat16
    # [core, chunk(8), p(128), s_global(20), (kh(2), k10(10)), d(3)]
    #   -> tile[p, kh, d, k10, (chunk, s_global)]
    def kp_shard(x):
        x = np.asarray(x, dtype=np.float32).reshape(
            N_CORES, 8, 128, S, 2, 10, 3).transpose(0, 2, 4, 6, 5, 1, 3)
        return np.ascontiguousarray(x).reshape(N_CORES, 128, 9600).astype(bf)
    pkh = kp_shard(pred_keypoints)
    gkh = kp_shard(gt_keypoints)
    pgh = np.stack([pkh, gkh], axis=1)          # [core, 2, 128, 9600]
    lg32 = np.asarray(pred_section_logits, dtype=np.float32)
    lgnh = lg32.reshape(N_CORES, NCH_CE, 128, T_CE, NS).transpose(0, 1, 2, 4, 3)
    lgnh = np.ascontiguousarray(lgnh).reshape(N_CORES, NCH_CE, 128, NS * T_CE)
    lgnh_hi = lgnh[:, :OH2X_CUT].astype(bf)
    lgnh_lo = np.ascontiguousarray(
        lgnh[:, OH2X_CUT:] if OH2X_CUT < NCH_CE else lgnh[:, :1]
    ).astype(ml_dtypes.float8_e4m3)
    lgfh = lg32.reshape(N_CORES, NCH_CE, FFL, 100).transpose(0, 1, 3, 2)
    lgfh = np.ascontiguousarray(lgfh).astype(ml_dtypes.float8_e4m3)
    lbh = np.asarray(gt_section_label).reshape(N_CORES, NCH_CE, 128, T_CE).astype(bf)
    ob = np.zeros((100, 5), dtype=np.float32)
    for g in range(5):
        ob[g * 20:(g + 1) * 20, g] = 1.0
    ob = ob.astype(bf)
    return [{"pg": pgh[i], "lgn": lgnh_hi[i], "lgn8": lgnh_lo[i],
             "lgf": lgfh[i], "lb": lbh[i], "ob": ob} for i in range(N_CORES)]


def combine_accs(accs):
    tot = np.zeros(NACC, dtype=np.float64)
    lse = 0.0
    for a in accs:
        a64 = a.astype(np.float64)
        tot += a64.sum(axis=0)
        for g in range(5):
            col = a64[:, C_LSE + g]
            for q in range(4):
                lse += col[32 * q:32 * q + 5].sum()
    ly = tot[C_LY:C_LY + NCH_CE].sum() + tot[C_LYX:C_LYX + 4].sum()
    kp = tot[C_KP]
    rot = tot[C_ROT]
    cent = tot[C_CENT]
    total = (1.0 * (lse - ly) / (B * K)
             + 4.0 * kp / (B * K * 3)
             + 5.0 * rot / (B * K * 3)
             + 6.0 * cent / (B * S * 3))
    return np.float32(total)


def kernel(**inputs) -> np.ndarray:
    nc = _build()
    in_maps = _shard_inputs(**inputs)
    res = run_bass_kernel_spmd(nc, in_maps, list(range(N_CORES))).results
    return combine_accs([res[i]["out"] for i in range(N_CORES)])


# revision 73
# speedup vs baseline: 1.0152x; 1.0152x over previous
"""Trainium2 Bass kernel for nn_KPLoss_377957122199 (v3, vector-lean).

loss = 1*CE + 4*smoothL1(kp) + 5*smoothL1(Procrustes rot residual)
     + 6*smoothL1(section-center diff)

Data-parallel over 8 cores (batch 8192 -> 1024/core). v3 design notes:
  * keypoints live in ONE [128, (d,k,s160)] bf16 tile per tensor (pred|gt
    packed side by side) so every phase-1/3 op is a full-width, long-run,
    2x-mode DVE op.  Products are emitted as two k-half tiles whose
    elementwise add IS tree level 1.
  * point sums (k-trees over raw keypoints) run on the idle GpSimd engine;
    phase-3's (gt + v) runs there too, overlapped with DVE products.
  * polar: 2 frob-scaled Newton iterations (optional plain Newton tail via
    TAIL_NEWTON; off -> rel ~7e-3 vs gate 2e-2, saves ~6.5us); reciprocals
    and powers go through batched Ln/Exp ACT ops in log domain, so the DVE
    does only small bf16 tensor ops.  Guards: |det| clamp, +-0.5 sign fold.
  * CE as in v2: fp8 logits shipped twice (n-major for the ONEHOT custom
    DVE gather; flat [100,4096] so a block-ones matmul reduces NS=20 on
    partitions into PSUM), ln reads PSUM packed 4 chunks/ACT op.
  * custom DVE ops: SL1_DIFF (fused smooth-L1-sum of (in0-in1)) and
    ONEHOT_DOT (sum_t logits[y_t,t] via PageIdx compare), 1 op per use.
"""

import sys
for _p in ("/opt/trn_rl_repo", "/root/.axon_site/_ro/trn_rl_repo"):
    if _p not in sys.path:
        sys.path.insert(0, _p)

from contextlib import ExitStack
from operator import add as _add_op

import numpy as np
import ml_dtypes

import concourse.bass as bass
import concourse.bacc as bacc
import concourse.mybir as mybir
import concourse.tile as tile
from concourse.bass_utils import run_bass_kernel_spmd

# ---- custom DVE ops (registered at import) --------------------------------
import concourse.dve_ops as dve_ops
from concourse.dve_ops import DveOp, OPS
from concourse.dve_spec import (
    C0, C1, C2, PageIdx, Spec, Src0, Src1, Zero,
    _has_src1, eq, lower, maxx, minn, select,
)
from concourse.dve_uop import DveOpSpec


def _sl1_ref(in0, in1, s0, s1, imm2):
    d = in0.astype(np.float32) - in1.astype(np.float32)
    t = np.clip(d, s0, s1)
    return (d - imm2 * t) * t


def _oh_ref(in0, in1, s0, s1, imm2):
    raise NotImplementedError


def _register(name, spec, subdim):
    if name in dve_ops._SUB_OPCODE_FOR_NAME:
        return next(o for o in OPS if o.name == name)
    row = dve_ops._CUSTOM_DVE_ROW_BASE + len(OPS)
    assert row < 0x20
    op = DveOp(name, spec, subdim=subdim, uops_sha={})
    for ver in ("v3", "v4"):
        s = DveOpSpec(name=name, opcode=row, uops=lower(spec, ver=ver),
                      rd1_en=_has_src1(spec))
        op.uops_sha[ver] = s.sha(ver)
    OPS.append(op)
    dve_ops._SUB_OPCODE_FOR_NAME[name] = row
    return op


_d = Src0 - Src1
_t = minn(maxx(_d, C0), C1)
SL1_DIFF = _register("SL1_DIFF", Spec(body=(_d - _t * C2) * _t, accum=_add_op,
                                      reference=_sl1_ref), subdim=False)
_pg = PageIdx(C0, C1)
ONEHOT_DOT = _register("ONEHOT_DOT",
                       Spec(body=select(eq(Src1, _pg), Src0, Zero),
                            accum=_add_op, reference=_oh_ref), subdim=True)

# ---- hand-written 2x_1p uop program for ONEHOT_DOT ------------------------
# The 1x program (3 states: init / steady / page-step) uses dp blocks 0-3:
#   dp0 carries the page counter (CURR_ALU_OUT temporal; +C1 in the step
#   state), dp1 IS_EQ(label, page), dp2 SELECT(zero, logit), dp3 adds the
#   selected value into the stage-local accumulator (CURR_ALU_OUT) and
#   captures the select into d0 for the WR0_LO output.
# The 2x variant processes the packed pair: extra input lanes route
# SRC_0_HI / SRC_1_HI; dp3/dp4 replicate IS_EQ/SELECT for the HI element,
# dp5 sums the LO+HI selects, dp6 accumulates the pair-sum, and WR0_HI
# emits the HI select from d1.  Lanes in the steady/step states:
#   d0: label_lo (consumed at dp1), then sel_lo from dp3 on
#   d1: zero, then sel_hi from dp5 on      d2: logit_lo
#   d3: C0 (dead), then logit_hi from dp0  d4: C1 (step), then page from dp1
#   d5: label_hi
ONEHOT_2X = True
TAIL_NEWTON = False             # 2 scaled iters only; rel ~7e-3 (gate 2e-2)


def _onehot_uops_2x(base):
    import copy
    from concourse.dve_uop import AluInp, DelayInp, InpSel, OutPath, OutSel
    from concourse.dve_spec import AluOp as _SpecAluOp  # noqa: F401
    from concourse.dve_uop import AluOp as _UopAluOp
    PP, PA = DelayInp.PREV_DELAY, DelayInp.PREV_ALU_OUT
    u2x = [copy.deepcopy(u) for u in base]
    for si in (1, 2):
        u = u2x[si]
        u.inp[0], u.inp_enable[0] = InpSel.SRC_0_HI, 1
        u.inp[6], u.inp_enable[6] = InpSel.SRC_1_HI, 1
        dp = u.datapath_config
        # dp0: page counter (op kept) + capture SRC_0_HI (lane0) into d3,
        # pass label_hi through d5
        dp[0].delay[3], dp[0].delay_enable[3] = PA, 1
        dp[0].delay[5], dp[0].delay_enable[5] = PP, 1
        # dp1: IS_EQ lo (kept) + capture page into d4
        dp[1].delay[4], dp[1].delay_enable[4] = PA, 1
        dp[1].delay[3], dp[1].delay_enable[3] = PP, 1
        dp[1].delay[5], dp[1].delay_enable[5] = PP, 1
        # dp2: SELECT lo (kept); keep lanes flowing
        dp[2].delay[3], dp[2].delay_enable[3] = PP, 1
        dp[2].delay[4], dp[2].delay_enable[4] = PP, 1
        dp[2].delay[5], dp[2].delay_enable[5] = PP, 1
        # dp3: accumulate sel_lo temporally (EXACTLY as the 1x program:
        # ADD(CURR_ALU_OUT, PREV_ALU_OUT), a-flop inject, capture sel_lo d0)
        # -- dp3 base is already that; also pass d1 (zero), d3 (logit_hi),
        # d4 (page), d5 (label_hi)
        dp[3].delay[1], dp[3].delay_enable[1] = PP, 1
        dp[3].delay[3], dp[3].delay_enable[3] = PP, 1
        dp[3].delay[4], dp[3].delay_enable[4] = PP, 1
        dp[3].delay[5], dp[3].delay_enable[5] = PP, 1
        # dp4: IS_EQ hi; capture acc_lo into d2
        dp[4].op = _UopAluOp.IS_EQ
        dp[4].alu_src0 = AluInp.PREV_DELAY_5
        dp[4].alu_src1 = AluInp.PREV_DELAY_4
        dp[4].delay[0], dp[4].delay_enable[0] = PP, 1
        dp[4].delay[1], dp[4].delay_enable[1] = PP, 1
        dp[4].delay[2], dp[4].delay_enable[2] = PA, 1
        dp[4].delay[3], dp[4].delay_enable[3] = PP, 1
        # dp5: SELECT hi
        dp[5].op = _UopAluOp.SELECT
        dp[5].alu_src0 = AluInp.PREV_DELAY_1
        dp[5].alu_src1 = AluInp.PREV_DELAY_3
        dp[5].delay[0], dp[5].delay_enable[0] = PP, 1
        dp[5].delay[2], dp[5].delay_enable[2] = PP, 1
        # dp6: accumulate sel_hi temporally; capture sel_hi into d1
        dp[6].op = _UopAluOp.ADD
        dp[6].alu_src0 = AluInp.CURR_ALU_OUT
        dp[6].alu_src1 = AluInp.PREV_ALU_OUT
        dp[6].delay[0], dp[6].delay_enable[0] = PP, 1
        dp[6].delay[1], dp[6].delay_enable[1] = PA, 1
        dp[6].delay[2], dp[6].delay_enable[2] = PP, 1
        # dp7: total = acc_lo + acc_hi.  The per-lane accumulator register is
        # not reachable from the 2x uop table (hw READ_ACCUMULATOR returns
        # junk), so the running total itself is emitted on WR0_LO: the last
        # even element of the dump holds the final sum.
        dp[7].op = _UopAluOp.ADD
        dp[7].alu_src0 = AluInp.PREV_DELAY_2
        dp[7].alu_src1 = AluInp.PREV_ALU_OUT
        dp[7].delay[0], dp[7].delay_enable[0] = PP, 1
        dp[7].delay[1], dp[7].delay_enable[1] = PP, 1
        u.out[OutPath.WR0_LO] = OutSel.ALU_OUT
        u.out_enable[OutPath.WR0_LO] = 1
        u.out[OutPath.WR0_HI] = OutSel.DELAY_1
        u.out_enable[OutPath.WR0_HI] = 1
    return u2x


def _install_onehot_2x():
    from concourse.dve_table_gen import dve_ver_for
    ver = dve_ver_for("TRN2")
    base = lower(ONEHOT_DOT.spec, ver=ver)
    spec = DveOpSpec(
        name="ONEHOT_DOT",
        opcode=dve_ops.get_dve_sub_opcode("ONEHOT_DOT"),
        uops=base,
        uops_2x=_onehot_uops_2x(base),
        perf_max=1,
        rd1_en=True,
    )
    for u in spec.uops_2x:
        u.validate(ver)
    dve_ops._COMPILE_CACHE[("ONEHOT_DOT", ver)] = spec


if ONEHOT_2X:
    _install_onehot_2x()

FP32 = mybir.dt.float32
BF16 = mybir.dt.bfloat16
FP8 = mybir.dt.float8e4
AX = mybir.AxisListType
OP = mybir.AluOpType
AF = mybir.ActivationFunctionType

N_CORES = 8
B, K, NS, SEC = 8192, 400, 20, 20
S = K // SEC                    # 20 sections / sample
BC = B // N_CORES               # 1024 samples / core
F = 160                         # sections per partition ("sall" = (chunk, s))
NCH_CE = 20                     # CE chunks
TOKC = BC * K // NCH_CE         # 20480 tokens / CE chunk
T_CE = TOKC // 128              # 160 tokens / partition (n-major layout)
FFL = TOKC * NS // 100          # 4096 cols in flat [100, .] layout

N_ITER = 2                      # scaled-Newton polar iterations
OH2X_CUT = 20                   # chunks >= this use fp8 + 1x ONEHOT
LN_HALF = float(np.log(0.5))
LN_DET_MIN = float(np.log(1e-6))

# acc column map
C_LSE = 0                       # 5 cols (groups of 4 chunks; rows 32q+0..4)
C_LY = C_LSE + 5                # 20 cols
C_KP = C_LY + NCH_CE            # 1
C_ROT = C_KP + 1                # 1
C_CENT = C_ROT + 1              # 1
C_LYX = C_CENT + 1              # 4 (second halves of split chunks 0..3)
NACC = C_LYX + 4


def _emit(ctx, tc, aps):
    nc = tc.nc
    pg, lgn, lgn8, lgf, lb, ob, out = (aps[k] for k in
                                 ("pg", "lgn", "lgn8", "lgf", "lb", "ob", "out"))

    pers = ctx.enter_context(tc.tile_pool(name="pers", bufs=1))
    scr = ctx.enter_context(tc.tile_pool(name="scr", bufs=1))
    cep = tc.alloc_tile_pool(name="ce", bufs=3)
    exp_pool = tc.alloc_tile_pool(name="exp", bufs=2)
    psp = tc.alloc_tile_pool(name="ps", bufs=1, space="PSUM")

    acc = pers.tile([128, NACC], FP32, tag="acc", name="acc")
    oneblk = pers.tile([100, 5], BF16, tag="oneblk", name="oneblk")
    nc.sync.dma_start(oneblk[:], ob)
    lnhalf = pers.tile([128, 1], FP32, tag="lnhalf", name="lnhalf")
    nc.gpsimd.memset(lnhalf[:], LN_HALF)

    # keypoints: pred at [:, 0:9600], gt at [:, 9600:19200]; layout (d,k,f)
    pgb = pers.tile([128, 2 * 9600], BF16, tag="pgb", name="pgb")
    p_all = pgb[:, 0:9600]
    g_all = pgb[:, 9600:19200]

    # A66 holds the polar 5x5-duplicated state; H == its X quadrant, so the
    # phase-1 tree writes land directly where the polar iteration reads.
    A66 = pers.tile([128, 25 * F], BF16, tag="A66", name="A66")
    A = A66[:].rearrange("p (a b f) -> p a b f", a=5, b=5)
    Hv = A[:, 0:3, 0:3]                                  # [128,3,3,F]
    spg = pers.tile([128, 2 * 3 * F], BF16, tag="spg", name="spg")  # [s,d,f] sums
    spgs = pers.tile([128, 2 * 3 * F], BF16, tag="spgs", name="spgs")  # /SEC
    R = pers.tile([128, 9 * F], BF16, tag="R", name="R")        # [i,j,f]
    v = pers.tile([128, 3 * F], BF16, tag="v", name="v")        # [j,f]

    # ---------------- cross entropy ----------------
    psum = psp.tile([128, FFL], FP32, tag="mm", name="mm")

    def ce_chunk(c):
        # early chunks: bf16 logits + 2x ONEHOT (fast, DVE-bound window);
        # late chunks: fp8 logits + 1x ONEHOT (halves their DMA in the
        # DMA-feed-bound tail, where the DVE idles anyway)
        two_x = ONEHOT_2X and c < OH2X_CUT
        split = two_x and c < 2          # class-halved gather: earlier start
        if two_x:
            lgnc = cep.tile([128, NS * T_CE], BF16, tag="lgn", name="lgn")
            if split:
                he = (NS // 2) * T_CE
                nc.sync.dma_start(lgnc[:, 0:he], lgn[c][:, 0:he])
                nc.sync.dma_start(lgnc[:, he:2 * he], lgn[c][:, he:2 * he])
            else:
                nc.sync.dma_start(lgnc[:], lgn[c])
        else:
            lgnc = cep.tile([128, NS * T_CE], FP8, tag="lgn8", name="lgn8")
            nc.sync.dma_start(lgnc[:], lgn8[c - OH2X_CUT])
        lbc = cep.tile([128, T_CE], BF16, tag="lbc", name="lbc")
        nc.sync.dma_start(lbc[:], lb[c])
        lgfc = cep.tile([100, FFL], FP8, tag="lgf", name="lgf")
        nc.sync.dma_start(lgfc[:], lgf[c])

        # l_y: one custom op.  In 2x mode the op streams its running total on
        # WR0_LO; the final sum sits in the last even dump element and a tiny
        # copy moves it into the acc column.
        dmp = scr.tile([128, NS * T_CE], BF16, tag="dmpce", name="dmpce")
        if split:
            he = (NS // 2) * T_CE
            for h in range(2):
                col = C_LY + c if h == 0 else C_LYX + c
                oh = nc.vector._custom_dve(
                    ONEHOT_DOT,
                    out=dmp[:, h * he:(h + 1) * he].rearrange(
                        "p (n t) -> p n t", n=NS // 2),
                    in0=lgnc[:, h * he:(h + 1) * he].rearrange(
                        "p (n t) -> p n t", n=NS // 2),
                    in1=lbc[:].unsqueeze(1).broadcast_to(
                        [128, NS // 2, T_CE]),
                    s0=float(h * (NS // 2)), s1=1.0, accum_out=None)
                oh.ins.perf_max = 1
                nc.vector.tensor_copy(acc[:, col:col + 1],
                                      dmp[:, (h + 1) * he - 2:(h + 1) * he - 1])
        else:
            oh = nc.vector._custom_dve(
                ONEHOT_DOT,
                out=dmp[:].rearrange("p (n t) -> p n t", n=NS),
                in0=lgnc[:].rearrange("p (n t) -> p n t", n=NS),
                in1=lbc[:].unsqueeze(1).broadcast_to([128, NS, T_CE]),
                s0=0.0, s1=1.0,
                accum_out=None if two_x else acc[:, C_LY + c:C_LY + c + 1])
            if two_x:
                oh.ins.perf_max = 1
                nc.vector.tensor_copy(acc[:, C_LY + c:C_LY + c + 1],
                                      dmp[:, NS * T_CE - 2:NS * T_CE - 1])

        # lse: exp (scalar) -> block-ones matmul (PE) -> ln on packed PSUM
        ex = exp_pool.tile([100, FFL], BF16, tag="ex", name="ex")
        nc.scalar.activation(ex[:], lgfc[:], AF.Exp)
        q = c % 4
        for h in range(FFL // 512):
            nc.tensor.matmul(
                psum[32 * q:32 * q + 5, h * 512:(h + 1) * 512],
                oneblk[:], ex[:, h * 512:(h + 1) * 512],
                start=True, stop=True, tile_position=(0, 32 * q))
        if q == 3:
            g = c // 4
            lnd = scr.tile([101, FFL], BF16, tag="lnd", name="lnd")
            nc.scalar.activation(lnd[:], psum[0:101, :], AF.Ln,
                                 accum_out=acc[0:101, C_LSE + g:C_LSE + g + 1])

    # ---------------- phase 1: keypoints ----------------
    def p1():
        wk = tc.alloc_tile_pool(name="wk1", bufs=1)
        # keypoints ship k-half-major: region layout (kh2, d3, k10, f160), so
        # the first product op depends only on the first half of the DMA.
        p4 = p_all.rearrange("p (kh d kf) -> p kh d kf", kh=2, d=3)
        g4 = g_all.rearrange("p (kh d kf) -> p kh d kf", kh=2, d=3)

        # products per k-half, each tree-reduced k10 -> k5 immediately, then
        # the two k5 tiles merge.  A2a ends up holding sum over each k-5-group.
        A2a = wk.tile([128, 9 * 5 * F], BF16, tag="A2a", name="A2a")
        A2av = A2a[:].rearrange("p (m k f) -> p m k f", m=9, k=5)
        A2b = wk.tile([128, 9 * 5 * F], BF16, tag="A2b", name="A2b")
        A2bv = A2b[:].rearrange("p (m k f) -> p m k f", m=9, k=5)
        for half, A2h in ((0, A2av), (1, A2bv)):
            T = wk.tile([128, 9 * 1600], BF16, tag="TT", name="TT")
            nc.vector.tensor_tensor(
                T[:].rearrange("p (i j kf) -> p i j kf", i=3, j=3),
                g4[:, half].unsqueeze(2).broadcast_to([128, 3, 3, 10 * F]),
                p4[:, half].unsqueeze(1).broadcast_to([128, 3, 3, 10 * F]),
                OP.mult)
            Tv = T[:].rearrange("p (m k f) -> p m k f", m=9, k=10)
            nc.vector.tensor_tensor(A2h, Tv[:, :, 0:5], Tv[:, :, 5:10], OP.add)
        nc.vector.tensor_tensor(A2a[:], A2a[:], A2b[:], OP.add)  # k5 sums
        D1 = wk.tile([128, 9 * 2 * F], BF16, tag="D1", name="D1")
        D1v = D1[:].rearrange("p (m k f) -> p m k f", m=9, k=2)
        nc.vector.tensor_tensor(D1v, A2av[:, :, 0:2], A2av[:, :, 2:4], OP.add)
        D2 = wk.tile([128, 9 * F], BF16, tag="D2", name="D2")
        D2v = D2[:].rearrange("p (m f) -> p m f", m=9)
        nc.vector.tensor_tensor(D2v, D1v[:, :, 0], D1v[:, :, 1], OP.add)
        nc.vector.tensor_tensor(
            Hv, D2[:].rearrange("p (i j f) -> p i j f", i=3, j=3),
            A2a[:].rearrange("p (i j k f) -> p i j k f", i=3, j=3, k=5)[:, :, :, 4],
            OP.add)

        # point sums over k for pred AND gt (k-tree, both tensors per op);
        # level 1 sums the two k-halves elementwise
        pgv = pgb[:].rearrange("p (s kh dkf) -> p s kh dkf", s=2, kh=2)
        PB1 = wk.tile([128, 9 * 1600], BF16, tag="TT", name="PB1")
        PB1v = PB1[:, 0:2 * 3 * 10 * F].rearrange(
            "p (s d k f) -> p s d k f", s=2, d=3, k=10)
        nc.vector.tensor_tensor(
            PB1v.rearrange("p s d k f -> p s (d k f)"),
            pgv[:, :, 0], pgv[:, :, 1], OP.add)
        PB2 = wk.tile([128, 2 * 3 * 5 * F], BF16, tag="PB2", name="PB2")
        PB2v = PB2[:].rearrange("p (s d k f) -> p s d k f", s=2, d=3, k=5)
        nc.vector.tensor_tensor(PB2v, PB1v[:, :, :, 0:5], PB1v[:, :, :, 5:10],
                                OP.add)
        PD1 = wk.tile([128, 2 * 3 * 2 * F], BF16, tag="PD1", name="PD1")
        PD1v = PD1[:].rearrange("p (s d k f) -> p s d k f", s=2, d=3, k=2)
        nc.vector.tensor_tensor(PD1v, PB2v[:, :, :, 0:2], PB2v[:, :, :, 2:4],
                                OP.add)
        PD2 = wk.tile([128, 2 * 3 * F], BF16, tag="PD2", name="PD2")
        PD2v = PD2[:].rearrange("p (s d f) -> p s d f", s=2, d=3)
        nc.vector.tensor_tensor(PD2v, PD1v[:, :, :, 0], PD1v[:, :, :, 1],
                                OP.add)
        nc.vector.tensor_tensor(
            spg[:].rearrange("p (s d f) -> p s d f", s=2, d=3),
            PD2v, PB2v[:, :, :, 4], OP.add)
        # scaled copies (sums/SEC) used by H-correction, v, and center loss
        nc.vector.tensor_scalar(spgs[:], spg[:], 1.0 / SEC, None, OP.mult)

        # H -= sg_i * (sp_j/SEC)
        spsv = spgs[:, 0:3 * F].rearrange("p (d f) -> p d f", d=3)
        sgv = spg[:, 3 * F:6 * F].rearrange("p (d f) -> p d f", d=3)
        Mt = wk.tile([128, 9 * 2 * F], BF16, tag="D1", name="M")
        M = Mt[:, 0:9 * F]
        nc.vector.tensor_tensor(
            M.rearrange("p (i j f) -> p i j f", i=3, j=3),
            sgv.unsqueeze(2).broadcast_to([128, 3, 3, F]),
            spsv.unsqueeze(1).broadcast_to([128, 3, 3, F]), OP.mult)
        nc.vector.tensor_tensor(
            Hv, Hv, M.rearrange("p (i j f) -> p i j f", i=3, j=3),
            OP.subtract)

        # keypoint smooth-L1 (one op) + center loss
        dmp = wk.tile([128, 9 * 1600], BF16, tag="TT", name="TTdmp")
        nc.vector._custom_dve(SL1_DIFF, out=dmp[:, 0:9600], in0=p_all,
                              in1=g_all, s0=-1.0, s1=1.0, imm2=0.5,
                              accum_out=acc[:, C_KP:C_KP + 1])
        dmpc = scr.tile([128, 3 * F], BF16, tag="dmpc", name="dmpc")
        nc.vector._custom_dve(SL1_DIFF, out=dmpc[:], in0=spgs[:, 0:3 * F],
                              in1=spgs[:, 3 * F:6 * F],
                              s0=-1.0, s1=1.0, imm2=0.5,
                              accum_out=acc[:, C_CENT:C_CENT + 1])
        wk.release()

    # ---------------- polar decomposition (batched 3x3, bf16) ----------------
    def polar(fillers=()):
        fillers = list(fillers)
        pol = tc.alloc_tile_pool(name="pol", bufs=1)
        X = Hv
        Cf = pol.tile([128, 9 * F], BF16, tag="cof", name="cof")
        C3v = Cf[:].rearrange("p (i j f) -> p i j f", i=3, j=3)
        T1 = pol.tile([128, 9 * F], BF16, tag="ct1", name="ct1")
        T2 = pol.tile([128, 9 * F], BF16, tag="ct2", name="ct2")
        SX = pol.tile([128, 9 * F], BF16, tag="sqX", name="sqX")
        q1 = pol.tile([128, 4 * F], BF16, tag="fq1", name="fq1")
        q2 = pol.tile([128, 2 * F], BF16, tag="fq2", name="fq2")
        P0 = pol.tile([128, 3 * F], BF16, tag="dp0", name="dp0")
        P0v = P0[:].rearrange("p (j f) -> p j f", j=3)
        det = pol.tile([128, F], BF16, tag="det", name="det")
        sgn05 = pol.tile([128, F], BF16, tag="sgn", name="sgn")
        # combined ACT operands: one Ln over [nx2 | nc2 | adet], one Exp over
        # [hzarg | twarg] -- 2 ACT round-trips per iteration instead of 5.
        lnin = pol.tile([128, 3 * F], BF16, tag="lnin", name="lnin")
        lnout = pol.tile([128, 3 * F], FP32, tag="lnout", name="lnout")
        exparg = pol.tile([128, 2 * F], FP32, tag="exparg", name="exparg")
        expout = pol.tile([128, 2 * F], BF16, tag="expout", name="expout")
        lnd = pol.tile([128, F], FP32, tag="lnd", name="lnd")
        t3 = pol.tile([128, F], FP32, tag="t3", name="t3")
        wz0 = pol.tile([128, F], BF16, tag="wz0", name="wz0")
        wz = pol.tile([128, F], BF16, tag="wz", name="wz")
        U1 = pol.tile([128, 9 * F], BF16, tag="u1", name="u1")
        U1v = U1[:].rearrange("p (i j f) -> p i j f", i=3, j=3)
        U2 = pol.tile([128, 9 * F], BF16, tag="u2", name="u2")
        U2v = U2[:].rearrange("p (i j f) -> p i j f", i=3, j=3)
        LN2 = float(np.log(2.0))
        chz = pers.tile([128, 1], FP32, tag="chz", name="chz")
        nc.gpsimd.memset(chz[:], -1.25 * LN2)

        def cof_det(adet_dst):
            # duplicate X -> A66 quadrants (on ACT), cofactors via shifted
            # views, det; adet = |det|/2 lands in adet_dst
            nc.scalar.copy(A[:, 0:3, 3:5], A[:, 0:3, 0:2])
            nc.scalar.copy(A[:, 3:5, :], A[:, 0:2, :])
            nc.vector.tensor_tensor(
                T1[:].rearrange("p (i j f) -> p i j f", i=3, j=3),
                A[:, 1:4, 1:4], A[:, 2:5, 2:5], OP.mult)
            nc.vector.tensor_tensor(
                T2[:].rearrange("p (i j f) -> p i j f", i=3, j=3),
                A[:, 1:4, 2:5], A[:, 2:5, 1:4], OP.mult)
            nc.vector.tensor_tensor(Cf[:], T1[:], T2[:], OP.subtract)
            nc.vector.tensor_tensor(P0v, A[:, 0, 0:3], C3v[:, 0], OP.mult)
            nc.vector.tensor_tensor(det[:].unsqueeze(1), P0v[:, 0:1],
                                    P0v[:, 1:2], OP.add)
            nc.vector.tensor_tensor(det[:], det[:], P0v[:, 2], OP.add)
            nc.vector.tensor_scalar(sgn05[:], det[:], 0.0, -0.5, OP.is_ge,
                                    OP.add)
            nc.vector.tensor_tensor(adet_dst, det[:], sgn05[:], OP.mult)

        def frob(dst, src4):
            # dst <- max(frob2(src4), 1e-12)   (a [128,F] slice AP)
            nc.vector.tensor_tensor(
                SX[:].rearrange("p (i j f) -> p i j f", i=3, j=3),
                src4, src4, OP.mult)
            sx = SX[:].rearrange("p (m f) -> p m f", m=9)
            q1v = q1[:].rearrange("p (m f) -> p m f", m=4)
            nc.vector.tensor_tensor(q1v, sx[:, 0:4], sx[:, 4:8], OP.add)
            q2v = q2[:].rearrange("p (m f) -> p m f", m=2)
            nc.vector.tensor_tensor(q2v, q1v[:, 0:2], q1v[:, 2:4], OP.add)
            nc.vector.tensor_tensor(dst.unsqueeze(1), q2v[:, 0:1],
                                    q2v[:, 1:2], OP.add)
            nc.vector.tensor_tensor(dst, dst, sx[:, 8], OP.add)
            nc.vector.tensor_scalar(dst, dst, 1e-12, None, OP.max)

        for it in range(N_ITER):
            frob(lnin[:, 0:F], X)    # overlaps the ACT dup-copies in cof_det
            cof_det(lnin[:, 2 * F:3 * F])
            frob(lnin[:, F:2 * F], C3v)
            nc.scalar.activation(lnout[:], lnin[:], AF.Ln)
            # lnd'' = max(ln(adet), LN_MIN-ln2) + ln2/2  (= clamped ln|det|
            # - ln2/2; the ln2 shifts from adet = |det|/2 fold into consts)
            nc.vector.tensor_scalar(lnd[:], lnout[:, 2 * F:3 * F],
                                    LN_DET_MIN - LN2, 0.5 * LN2,
                                    OP.max, OP.add)
            # t3'' = ln(nc2) - ln(nx2) - 2*lnd''
            nc.vector.tensor_tensor(t3[:], lnout[:, F:2 * F], lnout[:, 0:F],
                                    OP.subtract)
            nc.vector.scalar_tensor_tensor(t3[:], lnd[:], -2.0, t3[:],
                                           OP.mult, OP.add)
            # hzarg = t3''/4 - 1.25*ln2 ; twarg = -t3''/4 - lnd'' - ln2/4
            nc.vector.scalar_tensor_tensor(
                exparg[:, 0:F], t3[:], 0.25,
                chz[:].broadcast_to([128, F]), OP.mult, OP.add)
            nc.vector.scalar_tensor_tensor(exparg[:, F:2 * F], t3[:], -0.25,
                                           lnd[:], OP.mult, OP.subtract)
            nc.vector.tensor_scalar(exparg[:, F:2 * F], exparg[:, F:2 * F],
                                    -0.25 * LN2, None, OP.add)
            nc.scalar.activation(expout[:], exparg[:], AF.Exp)
            nc.vector.tensor_tensor(wz[:], expout[:, F:2 * F], sgn05[:],
                                    OP.mult)
            # X = X*hz + C*wz
            hzb = expout[:, 0:F].unsqueeze(1).unsqueeze(1).broadcast_to(
                [128, 3, 3, F])
            wzb = wz[:].unsqueeze(1).unsqueeze(1).broadcast_to([128, 3, 3, F])
            nc.vector.tensor_tensor(U1v, X, hzb, OP.mult)
            nc.vector.tensor_tensor(U2v, C3v, wzb, OP.mult)
            nc.vector.tensor_tensor(X, U1v, U2v, OP.add)

        Rv = R[:].rearrange("p (i j f) -> p i j f", i=3, j=3)
        if TAIL_NEWTON:
            # plain Newton tail: R = 0.5*X + C * sgn/(2|det|)
            adet_t = lnin[:, 2 * F:3 * F]
            cof_det(adet_t)
            nc.scalar.activation(lnd[:], adet_t, AF.Ln, scale=2.0)
            nc.vector.tensor_scalar(lnd[:], lnd[:], LN_DET_MIN, None, OP.max)
            nc.scalar.activation(wz0[:], lnd[:], AF.Exp, scale=-1.0)
            nc.vector.tensor_tensor(wz[:], wz0[:], sgn05[:], OP.mult)
            wzb = wz[:].unsqueeze(1).unsqueeze(1).broadcast_to(
                [128, 3, 3, F])
            nc.vector.tensor_tensor(U2v, C3v, wzb, OP.mult)
            nc.vector.tensor_scalar(Rv, X, 0.5, None, OP.mult)
            nc.vector.tensor_tensor(R[:], R[:], U2[:], OP.add)
        else:
            nc.vector.tensor_copy(Rv, X)

        # v_j = sum_i (sp_i/SEC) R_ij - sg_j/SEC
        spsv = spgs[:, 0:3 * F].rearrange("p (d f) -> p d f", d=3)
        sgsv = spgs[:, 3 * F:6 * F].rearrange("p (d f) -> p d f", d=3)
        Pv = pol.tile([128, 9 * F], BF16, tag="Pv", name="Pv")
        Pvv = Pv[:].rearrange("p (i j f) -> p i j f", i=3, j=3)
        nc.vector.tensor_tensor(
            Pvv, spsv.unsqueeze(2).broadcast_to([128, 3, 3, F]), Rv, OP.mult)
        vv = v[:].rearrange("p (j f) -> p j f", j=3)
        nc.vector.tensor_tensor(vv, Pvv[:, 0], Pvv[:, 1], OP.add)
        nc.vector.tensor_tensor(vv, vv, Pvv[:, 2], OP.add)
        nc.vector.tensor_tensor(vv, vv, sgsv, OP.subtract)
        pol.release()

    # ---------------- phase 3: rotation residual ----------------
    def p3_segments():
        """Phase 3 as a list of ~5us DVE segments; the schedule interleaves
        them between the DMA-paced tail CE chunks so each segment covers one
        chunk's lgn transfer latency."""
        wk = tc.alloc_tile_pool(name="wk3", bufs=1)
        # keypoint layout (kh2, d3, k10, f); T/gv tiles use (j, kh, k10, f)
        p3v = p_all.rearrange("p (kh d k f) -> p d kh k f", kh=2, d=3, k=10)
        g3v = g_all.rearrange("p (kh d k f) -> p d kh k f", kh=2, d=3, k=10)
        Rv = R[:].rearrange("p (i j f) -> p i j f", i=3, j=3)
        vv = v[:].rearrange("p (j f) -> p j f", j=3)
        gv = wk.tile([128, 9600], BF16, tag="gv", name="gv")
        gvv = gv[:].rearrange("p (j kh k f) -> p j kh k f", j=3, kh=2, k=10)
        T0 = wk.tile([128, 9600], BF16, tag="r0", name="r0")
        Tt = [T0,
              wk.tile([128, 9600], BF16, tag="r1", name="r1"),
              wk.tile([128, 9600], BF16, tag="r2", name="r2")]

        def seg_gv():
            for kh in range(2):
                nc.vector.tensor_tensor(
                    gvv[:, :, kh], g3v[:, :, kh],
                    vv.unsqueeze(2).broadcast_to([128, 3, 10, F]), OP.add)

        def seg_prod(i):
            def run():
                Tv = Tt[i][:].rearrange("p (j kh k f) -> p j kh k f",
                                        j=3, kh=2, k=10)
                for kh in range(2):
                    nc.vector.tensor_tensor(
                        Tv[:, :, kh],
                        p3v[:, i, kh].unsqueeze(0 + 1).broadcast_to(
                            [128, 3, 10, F]),
                        Rv[:, i].unsqueeze(2).broadcast_to([128, 3, 10, F]),
                        OP.mult)
            return run

        def seg_add(i):
            def run():
                nc.vector.tensor_tensor(T0[:], T0[:], Tt[i][:], OP.add)
            return run

        def seg_sl1():
            dmp = wk.tile([128, 9600], BF16, tag="r1", name="r1dmp")
            nc.vector._custom_dve(SL1_DIFF, out=dmp[:], in0=T0[:], in1=gv[:],
                                  s0=-1.0, s1=1.0, imm2=0.5,
                                  accum_out=acc[:, C_ROT:C_ROT + 1])
            wk.release()

        return [seg_gv, seg_prod(0), seg_prod(1), seg_prod(2),
                seg_add(1), seg_add(2), seg_sl1]

    # ---------------- schedule ----------------
    # keypoints first (4 DMAs for queue spread); CE chunk DMAs flow on other
    # queues.  Polar is emitted early so its small ACT ops (ln/exp) are not
    # queued behind the bulk of the 3.7us CE exp ops; the remaining CE
    # chunks fill DVE after polar while ACT drains exps, and phase 3 ends
    # the DVE stream.
    ce_chunk(0)
    ce_chunk(1)
    for h in range(2):
        nc.sync.dma_start(pgb[:, h * 4800:(h + 1) * 4800],
                          pg[0][:, h * 4800:(h + 1) * 4800])
        nc.sync.dma_start(pgb[:, 9600 + h * 4800:9600 + (h + 1) * 4800],
                          pg[1][:, h * 4800:(h + 1) * 4800])
    for c in range(2, 12):
        ce_chunk(c)
    p1()
    polar()
    for c in range(12, NCH_CE):
        ce_chunk(c)
    for seg in p3_segments():
        seg()
    psp.release()
    exp_pool.release()
    cep.release()
    nc.sync.dma_start(out[:], acc[:])


_CACHE = {}


def _build():
    if "nc" in _CACHE:
        return _CACHE["nc"]
    nc = bacc.Bacc("TRN2", target_bir_lowering=False, debug=False,
                   enable_asserts=False, num_devices=N_CORES)
    aps = {
        "pg": nc.dram_tensor("pg", [2, 128, 9600], BF16, kind="ExternalInput").ap(),
        "lgn": nc.dram_tensor("lgn", [OH2X_CUT, 128, NS * T_CE],
                              BF16, kind="ExternalInput").ap(),
        "lgn8": nc.dram_tensor("lgn8", [max(1, NCH_CE - OH2X_CUT), 128,
                                        NS * T_CE], FP8,
                               kind="ExternalInput").ap(),
        "lgf": nc.dram_tensor("lgf", [NCH_CE, 100, FFL], FP8, kind="ExternalInput").ap(),
        "lb": nc.dram_tensor("lb", [NCH_CE, 128, T_CE], BF16, kind="ExternalInput").ap(),
        "ob": nc.dram_tensor("ob", [100, 5], BF16, kind="ExternalInput").ap(),
        "out": nc.dram_tensor("out", [128, NACC], FP32, kind="ExternalOutput").ap(),
    }
    with tile.TileContext(nc) as tc:
        with ExitStack() as ctx:
            with nc.allow_low_precision(reason="bf16 tree reduces; validated"):
                _emit(ctx, tc, aps)
    nc.compile()
    _CACHE["nc"] = nc
    return nc


def _shard_inputs(pred_keypoints, gt_keypoints, pred_section_logits, gt_section_label):
    bf = ml_dtypes.bfloat16
    # [core, chunk(8), p(128), s_global(20), (kh(2), k10(10)), d(3)]
    #   -> tile[p, kh, d, k10, (chunk, s_global)]
    def kp_shard(x):
        x = np.asarray(x, dtype=np.float32).reshape(
            N_CORES, 8, 128, S, 2, 10, 3).transpose(0, 2, 4, 6, 5, 1, 3)
        return np.ascontiguousarray(x).reshape(N_CORES, 128, 9600).astype(bf)
    pkh = kp_shard(pred_keypoints)
    gkh = kp_shard(gt_keypoints)
    pgh = np.stack([pkh, gkh], axis=1)          # [core, 2, 128, 9600]
    lg32 = np.asarray(pred_section_logits, dtype=np.float32)
    lgnh = lg32.reshape(N_CORES, NCH_CE, 128, T_CE, NS).transpose(0, 1, 2, 4, 3)
    lgnh = np.ascontiguousarray(lgnh).reshape(N_CORES, NCH_CE, 128, NS * T_CE)
    lgnh_hi = lgnh[:, :OH2X_CUT].astype(bf)
    lgnh_lo = np.ascontiguousarray(
        lgnh[:, OH2X_CUT:] if OH2X_CUT < NCH_CE else lgnh[:, :1]
    ).astype(ml_dtypes.float8_e4m3)
    lgfh = lg32.reshape(N_CORES, NCH_CE, FFL, 100).transpose(0, 1, 3, 2)
    lgfh = np.ascontiguousarray(lgfh).astype(ml_dtypes.float8_e4m3)
    lbh = np.asarray(gt_section_label).reshape(N_CORES, NCH_CE, 128, T_CE).astype(bf)
    ob = np.zeros((100, 5), dtype=np.float32)
    for g in range(5):
        ob[g * 20:(g + 1) * 20, g] = 1.0
    ob = ob.astype(bf)
    return [{"pg": pgh[i], "lgn": lgnh_hi[i], "lgn8": lgnh_lo[i],
             "lgf": lgfh[i], "lb": lbh[i], "ob": ob} for i in range(N_CORES)]


def combine_accs(accs):
    tot = np.zeros(NACC, dtype=np.float64)
    lse = 0.0
    for a in accs:
        a64 = a.astype(np.float64)
        tot += a64.sum(axis=0)
        for g in range(5):
            col = a64[:, C_LSE + g]
            for q in range(4):
                lse += col[32 * q:32 * q + 5].sum()
    ly = tot[C_LY:C_LY + NCH_CE].sum() + tot[C_LYX:C_LYX + 4].sum()
    kp = tot[C_KP]
    rot = tot[C_ROT]
    cent = tot[C_CENT]
    total = (1.0 * (lse - ly) / (B * K)
             + 4.0 * kp / (B * K * 3)
             + 5.0 * rot / (B * K * 3)
             + 6.0 * cent / (B * S * 3))
    return np.float32(total)


def kernel(**inputs) -> np.ndarray:
    nc = _build()
    in_maps = _shard_inputs(**inputs)
    res = run_bass_kernel_spmd(nc, in_maps, list(range(N_CORES))).results
    return combine_accs([res[i]["out"] for i in range(N_CORES)])
